# revision 9
# baseline (speedup 1.0000x reference)
"""Distributed causal multi-head attention for 8 TRN2 NeuronCores.

Problem: B=2, T=2048, D=1024, H=16 heads (hd=64), f32 in/out.

Sharding: core i handles batch b=i//4 and head-group g=i%4 (4 heads).
Wq/Wk/Wv column-sharded ([1024, 256] per core), Wo row-sharded
([256, 1024] per core).  Each core computes a partial output projection
for its 4 heads over the full sequence; the host sums the 4 partials
per batch (the unshard step replaces the all-reduce).

Per-core dataflow (matmuls bf16 on TensorEngine, f32 accumulation):
  x --DMA--> SBUF --cast--> bf16 --DMA-xbar-transpose--> xT
  QT,KT [256(d),2048(t)] = W^T @ x^T   (d on partitions)
  V     [2048(t),256(d)]               (t on partitions, +ones col)
  ST[k,q] = K . Q^T  -> exp (ACT, scale=1/sqrt(64)) -> PT bf16
  causal: diagonal tiles narrowed to their valid q range; only the
  128-wide diagonal block needs an affine_select mask (gpsimd)
  AV (V stationary): avb[65, q] += Vaug[k,65]^T @ PT[k, q]
    row 64 = softmax denominator (ones column of Vaug)
  per q-tile: transpose avb chunk -> [128(q), 65], reciprocal of col
  64, normalize (DVE per-partition scalar) -> attn[q, dv]
  attnT via PE transpose -> out_partial[t,e] = attnT^T @ Wo

Emission is software-pipelined: scores of head-pair p interleave with
AV of pair p-1; each q-slab's epilogue (transpose + out-proj + DMA) is
emitted as soon as the slab completes, keeping the PE stream dense.
"""

import numpy as np

import concourse.bass as bass
import concourse.mybir as mybir
import concourse.tile as tile
from concourse import bacc
from concourse.bass_utils import run_bass_kernel_spmd
from concourse.masks import make_identity

F32 = mybir.dt.float32
BF16 = mybir.dt.bfloat16
AF = mybir.ActivationFunctionType

T = 2048  # sequence length
D = 1024  # embed dim
NH = 4  # heads per core
HD = 64  # head dim
DH = NH * HD  # 256, sharded d per core
TT = T // 128  # 16 t tiles
DT = D // 128  # 8 embed tiles
NSLAB = 4  # q slabs of 512
SCALE = 1.0 / np.sqrt(HD)

_NC_CACHE = None


def build():
    nc = bacc.Bacc(None, target_bir_lowering=False, debug=False)

    x = nc.declare_dram_parameter("x", [T, D], F32, isOutput=False)
    wq = nc.declare_dram_parameter("Wq", [D, DH], F32, isOutput=False)
    wk = nc.declare_dram_parameter("Wk", [D, DH], F32, isOutput=False)
    wv = nc.declare_dram_parameter("Wv", [D, DH], F32, isOutput=False)
    wo = nc.declare_dram_parameter("Wo", [DH, D], F32, isOutput=False)
    out = nc.declare_dram_parameter("out", [T, D], F32, isOutput=True)

    with tile.TileContext(nc) as tc:
        with (
            tc.tile_pool(name="persist", bufs=1) as persist,
            tc.tile_pool(name="xstage", bufs=5) as xstage_pool,
            tc.tile_pool(name="xbf", bufs=4) as xbf_pool,
            tc.tile_pool(name="wstage", bufs=2) as wstage_pool,
            tc.tile_pool(name="pt", bufs=2) as pt_pool,
            tc.tile_pool(name="avstg", bufs=2) as avstg_pool,
            tc.tile_pool(name="opev", bufs=2) as opev_pool,
            tc.tile_pool(name="recip", bufs=4) as recip_pool,
            tc.tile_pool(name="ps_st", bufs=2, space="PSUM") as ps_st,
            tc.tile_pool(name="ps_avb", bufs=2, space="PSUM") as ps_avb,
            tc.tile_pool(name="ps_n", bufs=2, space="PSUM") as ps_n,
        ):
            # ---- persistent SBUF tensors (distinct tags -> own slots) ----
            def P(shape, dtype, name):
                return persist.tile(shape, dtype, name=name, tag=name)

            ident_b = P([128, 128], BF16, "ident_b")
            make_identity(nc, ident_b)

            # weights, bf16: w*_bf[:, dt*256:(dt+1)*256] is D-tile dt
            wq_bf = P([128, DT * DH], BF16, "wq_bf")
            wk_bf = P([128, DT * DH], BF16, "wk_bf")
            wv_bf = P([128, DT * DH], BF16, "wv_bf")
            # wo_bf[:, i*1024:(i+1)*1024] is hdv-tile i
            wo_bf = P([128, 2 * D], BF16, "wo_bf")
            # xT[:, dt*2048 + t]: x transposed, bf16
            xT = P([128, DT * T], BF16, "xT")
            # QT/KT[:, m*2048 + t]: head h in tile h//2, rows (h%2)*64..+64
            QT = P([128, 2 * T], BF16, "QT")
            KT = P([128, 2 * T], BF16, "KT")
            # V with ones column: slice (tt, h) = [:, (tt*NH+h)*65 : +65]
            vbuf = P([128, TT * NH * 65], BF16, "vbuf")
            # attention output, natural: (qt, h) = [:, qt*256 + h*64]
            attn = P([128, TT * DH], BF16, "attn")
            # attn transposed: (i, t) = [:, i*2048 + t]
            attnT = P([128, 2 * T], BF16, "attnT")

            # ---- load + cast weights (casts on DVE) ----
            for w_ext, w_bf in ((wq, wq_bf), (wk, wk_bf), (wv, wv_bf)):
                for dt_ in range(DT):
                    ws = wstage_pool.tile([128, D], F32, name="ws")
                    nc.scalar.dma_start(
                        out=ws[:, 0:DH], in_=w_ext[dt_ * 128 : (dt_ + 1) * 128, :]
                    )
                    nc.vector.tensor_copy(
                        w_bf[:, dt_ * DH : (dt_ + 1) * DH], ws[:, 0:DH]
                    )
            for i in range(2):
                ws = wstage_pool.tile([128, D], F32, name="ws")
                nc.scalar.dma_start(out=ws[:], in_=wo[i * 128 : (i + 1) * 128, :])
                nc.vector.tensor_copy(wo_bf[:, i * D : (i + 1) * D], ws[:])

            # ones columns of vbuf (col 64 of each 65-block)
            vb3 = vbuf.rearrange("p (t c) -> p t c", c=65)
            nc.gpsimd.memset(vb3[:, :, 64:65], 1.0)

            xT3 = xT.rearrange("p (d t) -> p d t", d=DT)

            def emit_x_tiles(tts):
                # load f32, cast to bf16 (ACT), PE-transpose (4 per bank)
                for tt in tts:
                    xs = xstage_pool.tile([128, D], F32, name="xs")
                    eng = nc.sync if tt % 2 == 0 else nc.scalar
                    eng.dma_start(
                        out=xs[:], in_=x[tt * 128 : (tt + 1) * 128, :]
                    )
                    xb = xbf_pool.tile([128, D], BF16, name="xb")
                    nc.scalar.copy(out=xb[:], in_=xs[:])
                    for g4 in range(2):
                        ps = ps_n.tile([128, 512], BF16, name="psxt", tag="psn")
                        for u in range(4):
                            dt_ = g4 * 4 + u
                            nc.tensor.transpose(
                                ps[:, u * 128 : (u + 1) * 128],
                                xb[:, dt_ * 128 : (dt_ + 1) * 128],
                                ident_b[:],
                            )
                        nc.vector.tensor_copy(
                            xT3[:, g4 * 4 : (g4 + 1) * 4, tt * 128 : (tt + 1) * 128],
                            ps.rearrange("p (u c) -> p u c", u=4),
                        )

            def emit_qk_proj(ch2):
                # QT/KT columns [ch2*1024, +1024]
                for w_bf, outT in ((wq_bf, QT), (wk_bf, KT)):
                    for m in range(2):
                        ps = ps_st.tile([128, 1024], F32, name="psst")
                        for dt_ in range(DT):
                            lhsT = w_bf[
                                :, dt_ * DH + m * 128 : dt_ * DH + (m + 1) * 128
                            ]
                            for half in range(2):
                                c0 = ch2 * 1024 + half * 512
                                nc.tensor.matmul(
                                    ps[:, half * 512 : (half + 1) * 512],
                                    lhsT=lhsT,
                                    rhs=xT[:, dt_ * T + c0 : dt_ * T + c0 + 512],
                                    start=(dt_ == 0),
                                    stop=(dt_ == DT - 1),
                                )
                        nc.vector.tensor_copy(
                            outT[:, m * T + ch2 * 1024 : m * T + (ch2 + 1) * 1024],
                            ps[:],
                        )

            vb4 = vbuf.rearrange("p (n c) -> p n c", c=65)

            def emit_v_proj(tts):
                for tt in tts:
                    ps = ps_avb.tile([128, 256], F32, name="psavb", tag="psavb")
                    for dt_ in range(DT):
                        nc.tensor.matmul(
                            ps[:],
                            lhsT=xT[:, dt_ * T + tt * 128 : dt_ * T + (tt + 1) * 128],
                            rhs=wv_bf[:, dt_ * DH : (dt_ + 1) * DH],
                            start=(dt_ == 0),
                            stop=(dt_ == DT - 1),
                        )
                    nc.vector.tensor_copy(
                        vb4[:, tt * NH : (tt + 1) * NH, 0:64],
                        ps.rearrange("p (n c) -> p n c", n=NH),
                    )

            # ---- attention emission helpers ----
            def scores_chunks(s, h, pt):
                """List of thunks; each computes scores+exp for 1-2 k-tiles."""
                m, r0 = h // 2, (h % 2) * 64

                def off_diag(kt):
                    def go():
                        ps = ps_st.tile([128, 1024], F32, name="psst")
                        for u in range(2):
                            nc.tensor.matmul(
                                ps[:, u * 512 : (u + 1) * 512],
                                lhsT=KT[
                                    r0 : r0 + 64,
                                    m * T + (kt + u) * 128 : m * T + (kt + u + 1) * 128,
                                ],
                                rhs=QT[
                                    r0 : r0 + 64,
                                    m * T + s * 512 : m * T + (s + 1) * 512,
                                ],
                                start=True,
                                stop=True,
                            )
                        nc.scalar.activation(
                            out=pt[:, kt * 512 : (kt + 2) * 512],
                            in_=ps[:],
                            func=AF.Exp,
                            scale=float(SCALE),
                        )

                    return go

                def diag(j):
                    kt = 4 * s + j
                    n = 512 - 128 * j

                    def go():
                        ps = ps_st.tile([128, 1024], F32, name="psst")
                        nc.tensor.matmul(
                            ps[:, 0:n],
                            lhsT=KT[
                                r0 : r0 + 64, m * T + kt * 128 : m * T + (kt + 1) * 128
                            ],
                            rhs=QT[
                                r0 : r0 + 64,
                                m * T + s * 512 + 128 * j : m * T + (s + 1) * 512,
                            ],
                            start=True,
                            stop=True,
                        )
                        nc.scalar.activation(
                            out=pt[:, kt * 512 + 128 * j : (kt + 1) * 512],
                            in_=ps[:, 0:n],
                            func=AF.Exp,
                            scale=float(SCALE),
                        )
                        # mask the 128-wide diagonal block: keep qc_local>=kr
                        nc.gpsimd.affine_select(
                            out=pt[:, kt * 512 + 128 * j : kt * 512 + 128 * (j + 1)],
                            in_=pt[:, kt * 512 + 128 * j : kt * 512 + 128 * (j + 1)],
                            pattern=[[1, 128]],
                            compare_op=mybir.AluOpType.is_ge,
                            fill=0.0,
                            base=0,
                            channel_multiplier=-1,
                        )

                    return go

                return [off_diag(2 * u) for u in range(2 * s)] + [
                    diag(j) for j in range(4)
                ]

            def av_ops(s, h, pt):
                """Thunks: V-stationary AV accumulation, then per-q-tile
                transpose + normalize."""
                nk = 4 * (s + 1)
                stg = {}

                def av_go():
                    avb = ps_avb.tile([128, 512], F32, name="psavb", tag="psavb")
                    for kt in range(nk):
                        j = kt - 4 * s
                        off = 128 * j if j > 0 else 0
                        nc.tensor.matmul(
                            avb[0:65, off:512],
                            lhsT=vb4[:, kt * NH + h, :],
                            rhs=pt[:, kt * 512 + off : (kt + 1) * 512],
                            start=(kt == 0),
                            stop=(kt == nk - 1),
                        )
                    st = avstg_pool.tile([65, 512], BF16, name="avst")
                    stg["st"] = st
                    nc.vector.tensor_copy(st[:], avb[0:65, :])

                pnst = {}

                def tr_go(qi):
                    def go():
                        st = stg["st"]
                        if qi == 0:
                            pnst["pn"] = ps_n.tile(
                                [128, 264], BF16, name="psn", tag="psn"
                            )
                        pn = pnst["pn"]
                        nc.tensor.transpose(
                            pn[:, qi * 66 : qi * 66 + 65],
                            st[:, qi * 128 : (qi + 1) * 128],
                            ident_b[0:65, 0:65],
                        )
                        if qi == 3:
                            rc = recip_pool.tile([128, 4], F32, name="rc")
                            pnst["rc"] = rc
                            nc.vector.reciprocal(
                                rc[:],
                                pn.rearrange("p (n c) -> p n c", c=66)[:, :, 64],
                            )

                    return go

                def norm_go(qi):
                    def go():
                        qt = 4 * s + qi
                        pn, rc = pnst["pn"], pnst["rc"]
                        nc.vector.tensor_scalar_mul(
                            attn[:, qt * DH + h * 64 : qt * DH + (h + 1) * 64],
                            pn[:, qi * 66 : qi * 66 + 64],
                            rc[:, qi : qi + 1],
                        )

                    return go

                return (
                    [av_go]
                    + [tr_go(qi) for qi in range(4)]
                    + [norm_go(qi) for qi in range(4)]
                )

            at3 = attnT.rearrange("p (i t) -> p i t", i=2)

            def emit_slab_epilogue(s):
                for qt in range(4 * s, 4 * (s + 1)):
                    ps = ps_n.tile([128, 256], BF16, name="psnb", tag="psn")
                    for i in range(2):
                        nc.tensor.transpose(
                            ps[:, i * 128 : (i + 1) * 128],
                            attn[:, qt * DH + i * 128 : qt * DH + (i + 1) * 128],
                            ident_b[:],
                        )
                    nc.vector.tensor_copy(
                        at3[:, :, qt * 128 : (qt + 1) * 128],
                        ps.rearrange("p (i c) -> p i c", i=2),
                    )
                for tt in range(4 * s, 4 * (s + 1)):
                    ps = ps_st.tile([128, 1024], F32, name="psst")
                    for i in range(2):
                        lhsT = attnT[:, i * T + tt * 128 : i * T + (tt + 1) * 128]
                        for ec in range(2):
                            nc.tensor.matmul(
                                ps[:, ec * 512 : (ec + 1) * 512],
                                lhsT=lhsT,
                                rhs=wo_bf[:, i * D + ec * 512 : i * D + (ec + 1) * 512],
                                start=(i == 0),
                                stop=(i == 1),
                            )
                    ev = opev_pool.tile([128, 1024], F32, name="ev")
                    nc.vector.tensor_copy(ev[:], ps[:])
                    nc.sync.dma_start(
                        out=out[tt * 128 : (tt + 1) * 128, :], in_=ev[:]
                    )

            def interleave(a, b):
                """Merge op lists proportionally (a paced, b filled in)."""
                if not a:
                    return list(b)
                if not b:
                    return list(a)
                res = []
                nb, na, bi = len(b), len(a), 0
                for i, op in enumerate(a):
                    res.append(op)
                    want = (i + 1) * nb // na
                    while bi < want:
                        res.append(b[bi])
                        bi += 1
                res.extend(b[bi:])
                return res

            # ---- phase 0/1: x load/cast/transpose + projections ----
            emit_x_tiles(range(0, 8))
            emit_qk_proj(0)
            emit_x_tiles(range(8, 16))
            emit_qk_proj(1)
            emit_v_proj(range(0, TT))
            del emit_x_tiles

            # ---- attention, software-pipelined by one head-pair ----
            pairs = [(s, h) for s in range(NSLAB) for h in range(NH)]
            pts = {}
            prev = None
            for idx in range(len(pairs) + 1):
                sc = []
                if idx < len(pairs):
                    s, h = pairs[idx]
                    pts[idx] = pt_pool.tile([128, TT * 512], BF16, name="pt")
                    sc = scores_chunks(s, h, pts[idx])
                av = []
                if prev is not None:
                    ps_, ph_ = pairs[prev]
                    av = av_ops(ps_, ph_, pts[prev])
                for op in interleave(sc, av):
                    op()
                # epilogue for slab s once pair (s+1, 0)'s AV has been
                # emitted (one extra pair of delay keeps PE off ACT's back)
                if prev is not None:
                    ds_, dh_ = pairs[prev]
                    if dh_ == 0 and ds_ >= 1:
                        emit_slab_epilogue(ds_ - 1)
                prev = idx
            emit_slab_epilogue(NSLAB - 1)

    nc.compile()
    return nc


def _get_nc():
    global _NC_CACHE
    if _NC_CACHE is None:
        _NC_CACHE = build()
    return _NC_CACHE


def make_in_maps(x, Wq, Wk, Wv, Wo):
    x = np.asarray(x, dtype=np.float32)
    Wq = np.asarray(Wq, dtype=np.float32)
    Wk = np.asarray(Wk, dtype=np.float32)
    Wv = np.asarray(Wv, dtype=np.float32)
    Wo = np.asarray(Wo, dtype=np.float32)
    in_maps = []
    for core in range(8):
        b, g = core // 4, core % 4
        sl = slice(g * DH, (g + 1) * DH)
        in_maps.append(
            {
                "x": np.ascontiguousarray(x[b]),
                "Wq": np.ascontiguousarray(Wq[:, sl]),
                "Wk": np.ascontiguousarray(Wk[:, sl]),
                "Wv": np.ascontiguousarray(Wv[:, sl]),
                "Wo": np.ascontiguousarray(Wo[sl, :]),
            }
        )
    return in_maps


def unshard(results):
    out = np.empty((2, T, D), np.float32)
    for b in range(2):
        out[b] = results[4 * b]["out"]
        for g in range(1, 4):
            out[b] += results[4 * b + g]["out"]
    return out


def kernel(x, Wq, Wk, Wv, Wo):
    nc = _get_nc()
    in_maps = make_in_maps(x, Wq, Wk, Wv, Wo)
    res = run_bass_kernel_spmd(nc, in_maps, core_ids=list(range(8)))
    return unshard(res.results)
